# revision 30
# baseline (speedup 1.0000x reference)
"""Sparse-attention Trainium2 kernel, 8-way data-parallel over batch.

Reference computation (per batch):
  qkv = x @ qkv_w.T + qkv_b              -> split q,k,v [H=12, N=388, D=64]
  template queries (tokens 0:128) attend to template keys (0:128)
  search queries (tokens 128:388) attend to all 388 keys
  out = concat @ proj_w.T + proj_b

Kernel strategy per core (B_local=8 batches, all compute on device, bf16
matmuls with fp32 PSUM accumulation):
  - x cast to bf16, transposed feature-major via PE transposes.
  - q^T,k^T = W^T-stationary matmuls (feature-major out, per-partition bias
    added in fp32 on ACT/DVE during the PSUM->SBUF copy).
  - v = x^T-stationary matmuls (token-major out), stored per-head with a ones
    column appended so the attention-value matmul also produces softmax sums.
  - ALL remainder-token work (tokens 384:388 of each batch) is hoisted into
    shared 8-batch passes: one upfront pass builds the remainder x^T columns
    and remainder v for all 8 batches; one final pass projects all 8 batches'
    remainder tokens.  This removes the tiny per-batch matmuls whose
    dispatch+ldweights cost dominates their streaming time.
  - scores computed TRANSPOSED: S^T[k,q] = k^T-slices as lhsT, q^T as rhs.
    exp on ACT (scale=1/8 folded in), probs in bf16.
  - AV: out^T[d,q] accumulated over k-chunks; row 64 = softmax denominators.
    AV for head h is emitted during head h+1's scores so the exps are ready.
  - normalize (deferred, per HEAD PAIR): reciprocals (DVE, bf16) of both
    heads' denominator rows go into one [2,N] tile; a single [2,128] 0/1-mask
    PE matmul broadcasts head-even's recip to partitions 0:64 and head-odd's
    to 64:128, ACT-staged to SBUF, then two DVE multiplies normalize.
  - proj matmul reads attention output directly (no transposes), bias on DVE,
    DMA out token-major fp32.
  - weight prep: fp32 DMA, DVE cast to bf16 (prefetched 2 chunks ahead),
    1-cycle/row PE transposes.
"""

import numpy as np

B, N, C = 64, 388, 768
H, D = 12, 64
LT = 128          # template tokens (= first token chunk, exactly)
LS = N - LT       # 260 search tokens
NCORES = 8
BL = B // NCORES  # 8 batches per core
O3 = 3 * C        # 2304
SCALE = 0.125

_NC_CACHE = {}


def _build_nc(dump=False, reps=1, skip=()):
    from contextlib import ExitStack

    import concourse.tile as tile
    from concourse import bacc, mybir
    from concourse.masks import make_identity

    f32 = mybir.dt.float32
    bf16 = mybir.dt.bfloat16
    Identity = mybir.ActivationFunctionType.Identity
    Exp = mybir.ActivationFunctionType.Exp
    mult = mybir.AluOpType.mult
    add = mybir.AluOpType.add

    nc = bacc.Bacc("TRN2", target_bir_lowering=False)

    x_ext = nc.dram_tensor("x", [BL, N, C], f32, kind="ExternalInput")
    qkvw_ext = nc.dram_tensor("qkv_w", [O3, C], f32, kind="ExternalInput")
    qkvb_ext = nc.dram_tensor("qkv_b", [O3], f32, kind="ExternalInput")
    projw_ext = nc.dram_tensor("proj_w", [C, C], f32, kind="ExternalInput")
    projb_ext = nc.dram_tensor("proj_b", [C], f32, kind="ExternalInput")
    out_ext = nc.dram_tensor("out", [BL, N, C], f32, kind="ExternalOutput")

    # token chunking of the 388 tokens: 128,128,128 + 4 remainder (hoisted)
    TCH = [(0, 128), (128, 128), (256, 128), (384, 4)]
    TCH3 = TCH[0:3]

    with tile.TileContext(nc) as tc, ExitStack() as ctx:
        const = ctx.enter_context(tc.tile_pool(name="const", bufs=1))
        stage = ctx.enter_context(tc.tile_pool(name="stage", bufs=4))
        # per-rep shared tiles (remainder-token passes)
        rpool = ctx.enter_context(tc.tile_pool(name="rp", bufs=2))
        # 8 PSUM banks total: 5 general + 2 deferred-AV accumulators + 1
        # reciprocal-broadcast target
        psum = ctx.enter_context(tc.tile_pool(name="ps", bufs=5, space="PSUM"))
        pavpool = ctx.enter_context(tc.tile_pool(name="pav", bufs=2, space="PSUM"))
        pbcpool = ctx.enter_context(tc.tile_pool(name="pbc", bufs=1, space="PSUM"))

        ident = const.tile([128, 128], f32)
        make_identity(nc, ident)
        ident_b = const.tile([128, 128], bf16)
        make_identity(nc, ident_b)
        # head-pair normalize broadcast mask: row0 -> partitions 0:64,
        # row32 -> partitions 64:128 (engine writes must start at partition
        # 0/32/64/96, so the two reciprocals land on rows 0 and 32 of a
        # [33,N] tile; mask rows 1..31 are zero so those rows don't
        # contribute). Two ping-pong rinv tiles (memset once) avoid a
        # per-pair clear of the unused rows.
        mask33 = const.tile([33, 128], bf16)
        nc.vector.memset(mask33[:], 0.0)
        nc.vector.memset(mask33[0:1, 0:64], 1.0)
        nc.vector.memset(mask33[32:33, 64:128], 1.0)
        rinvA = const.tile([33, N], bf16)
        rinvB = const.tile([33, N], bf16)
        nc.vector.memset(rinvA[0:32, :], 0.0)
        nc.vector.memset(rinvB[0:32, :], 0.0)

        # ---- weights/biases declared here; emission interleaved with batch 0
        wT = const.tile([128, 6, O3], bf16)
        projT = const.tile([128, 6, C], bf16)
        qkb_sb = const.tile([128, 12], f32)
        vb_bc = const.tile([128, C], f32)
        pb_bc = const.tile([128, C], f32)

        def weights_gen():
            qb_st = stage.tile([12, 128], f32, tag="bst")
            nc.sync.dma_start(out=qb_st[:], in_=qkvb_ext[0:1536].rearrange("(j p) -> j p", p=128))
            pbt = psum.tile([128, 12], f32, tag="ps")
            nc.tensor.transpose(pbt[:], qb_st[:], ident[0:12, 0:12])
            nc.scalar.copy(out=qkb_sb[:], in_=pbt[:])

            # weight chunks: DMA fp32, cast bf16 on DVE, then 1-cycle/row PE
            # transposes. DMA+cast run two chunks ahead so the PE never waits
            # on the DMA->cast latency chain.
            wstbs = {}

            def fetch(j):
                wstf = stage.tile([128, C], f32, tag="wstf")
                src = qkvw_ext[j * 128:(j + 1) * 128, :] if j < 18 else \
                    projw_ext[(j - 18) * 128:(j - 17) * 128, :]
                nc.sync.dma_start(out=wstf[:], in_=src)
                wstb = stage.tile([128, C], bf16, tag="wstb")
                nc.vector.tensor_copy(out=wstb[:], in_=wstf[:])
                wstbs[j] = wstb

            fetch(0)
            fetch(1)
            for j in range(24):
                if j == 4:
                    nc.sync.dma_start(out=vb_bc[:], in_=qkvb_ext[1536:2304].unsqueeze(0).to_broadcast([128, C]))
                    nc.sync.dma_start(out=pb_bc[:], in_=projb_ext[:].unsqueeze(0).to_broadcast([128, C]))
                if j + 2 < 24:
                    fetch(j + 2)
                wstb = wstbs.pop(j)
                wdst = wT if j < 18 else projT
                jo = j * 128 if j < 18 else (j - 18) * 128
                # all 6 transposes into one bf16 PSUM bank, single copy out
                pt = psum.tile([128, 6 * 128], bf16, tag="ps")
                for ct in range(6):
                    nc.tensor.transpose(pt[:, ct * 128:(ct + 1) * 128],
                                        wstb[:, ct * 128:(ct + 1) * 128],
                                        ident_b[:])
                dst = wdst[:, :, jo:jo + 128]
                src_ap = pt[:].rearrange("p (g t) -> p g t", g=6)
                if j % 2 == 0:
                    nc.scalar.copy(out=dst, in_=src_ap)
                else:
                    nc.vector.tensor_copy(out=dst, in_=src_ap)
                yield

        # ---- per-rep remainder-token passes --------------------------------
        def vrem_partA(rst):
            """Remainder x rows for all 8 batches -> feature-major xT8
            [128, 6, 32] (32 = batch-major remainder tokens)."""
            xrem_f = rpool.tile([32, C], f32, tag="xrf")
            for b in range(BL):
                nc.sync.dma_start(out=xrem_f[b * 4:(b + 1) * 4, :],
                                  in_=x_ext[b, 384:388, :])
            xrem_c = rpool.tile([32, C], bf16, tag="xrc")
            nc.scalar.copy(out=xrem_c[:], in_=xrem_f[:])
            xT8 = rpool.tile([128, 6, 32], bf16, tag="xt8")
            rst["xT8"] = xT8
            pt = psum.tile([128, 192], bf16, tag="ps")
            for ct in range(6):
                nc.tensor.transpose(pt[:, ct * 32:(ct + 1) * 32],
                                    xrem_c[:, ct * 128:(ct + 1) * 128],
                                    ident_b[0:32, 0:32])
            nc.vector.tensor_copy(out=xT8[:],
                                  in_=pt[:].rearrange("p (g t) -> p g t", g=6))
            # allocate the rep's shared tiles here so stage1/proj_gen can
            # reference them before partB's matmuls are emitted
            vr8 = rpool.tile([32, H, 65], bf16, tag="vr8")
            prem8 = rpool.tile([128, 6, 32], bf16, tag="pr8")
            rst["vr8"] = vr8
            rst["prem8"] = prem8
            yield

        def vrem_partB(rst):
            """Remainder v (+ones col) for all 8 batches: vr8 [32, H, 65]."""
            xT8 = rst["xT8"]
            vr8 = rst["vr8"]
            nc.vector.memset(vr8[:, :, 64:65], 1.0)
            if "qkv" in skip:
                nc.vector.memset(vr8[:, :, 0:64], 0.0)
                return
            for o0, on, hs, he in ((0, 512, 0, 8), (512, 256, 8, 12)):
                pv = psum.tile([32, on], f32, tag="ps")
                for ct in range(6):
                    nc.tensor.matmul(
                        pv[:, 0:on],
                        lhsT=xT8[:, ct, :],
                        rhs=wT[:, ct, 1536 + o0:1536 + o0 + on],
                        start=(ct == 0), stop=(ct == 5),
                    )
                nc.vector.tensor_tensor(
                    out=vr8[:, hs:he, 0:64],
                    in0=pv[:].rearrange("p (h d) -> p h d", h=he - hs),
                    in1=vb_bc[0:32, o0:o0 + on].rearrange("p (h d) -> p h d", h=he - hs),
                    op=add,
                )
                yield

        # ---- per-batch pools ----
        xpool = ctx.enter_context(tc.tile_pool(name="xp", bufs=2))
        xtpool = ctx.enter_context(tc.tile_pool(name="xtp", bufs=2))
        qkpool = ctx.enter_context(tc.tile_pool(name="qkp", bufs=2))
        vpool = ctx.enter_context(tc.tile_pool(name="vp", bufs=2))
        apool = ctx.enter_context(tc.tile_pool(name="ap", bufs=2))
        ppool = ctx.enter_context(tc.tile_pool(name="pp", bufs=4))
        spool = ctx.enter_context(tc.tile_pool(name="ssp", bufs=4))
        opool = ctx.enter_context(tc.tile_pool(name="op", bufs=3))

        def emit_xload(b):
            # one DMA per 128-token chunk: the first transpose only waits for
            # chunk 0 (~1/3 of the full-x DMA time). Remainder rows 384:388
            # are handled by the shared vrem pass.
            xf = xpool.tile([128, 3, C], f32, tag="xf")
            for ti in range(3):
                nc.sync.dma_start(out=xf[:, ti, :],
                                  in_=x_ext[b, ti * 128:(ti + 1) * 128, :])
            return xf

        def stage1(b, xf, st, rst):
            """Generator: transposes (3 items), q/k groups (12), v (6).
            Yields between PE-work units so attention of the previous batch
            can interleave. Fills `st` with the batch's tiles."""
            xTb = xtpool.tile([128, 6, N], bf16, tag="xt")
            st["xT"] = xTb
            xc = xpool.tile([128, 3, C], bf16, tag="xc")
            for ti, (t0, tp) in enumerate(TCH3):
                # cast this chunk to bf16 on ACT, then 1-cycle/row transposes
                # (all 6 into one bf16 PSUM bank, single copy out)
                nc.scalar.copy(out=xc[0:tp, ti, :], in_=xf[0:tp, ti, :])
                pt = psum.tile([128, 6 * tp], bf16, tag="ps")
                for ct in range(6):
                    nc.tensor.transpose(pt[:, ct * tp:(ct + 1) * tp],
                                        xc[0:tp, ti, ct * 128:(ct + 1) * 128],
                                        ident_b[0:tp, 0:tp])
                dst = xTb[:, :, t0:t0 + tp]
                src_ap = pt[:].rearrange("p (g t) -> p g t", g=6)
                if ti % 2 == 0:
                    nc.vector.tensor_copy(out=dst, in_=src_ap)
                else:
                    nc.scalar.copy(out=dst, in_=src_ap)
                yield
            # remainder columns from the shared 8-batch pass
            nc.vector.tensor_copy(out=xTb[:, :, 384:388],
                                  in_=rst["xT8"][:, :, b * 4:(b + 1) * 4])

            qTb = qkpool.tile([128, 6, N], bf16, tag="q")
            kTb = qkpool.tile([128, 6, N], bf16, tag="k")
            st["q"], st["k"] = qTb, kTb
            if "qkv" in skip:
                nc.vector.memset(qTb[:, 0:1, 0:2], 0.0)
                nc.vector.memset(kTb[:, 0:1, 0:2], 0.0)
            for j in range(12 if "qkv" not in skip else 0):
                ps = psum.tile([128, N], f32, tag="ps")
                for ct in range(6):
                    nc.tensor.matmul(
                        ps[:],
                        lhsT=wT[:, ct, j * 128:(j + 1) * 128],
                        rhs=xTb[:, ct, :],
                        start=(ct == 0), stop=(ct == 5),
                    )
                dst = qTb[:, j, :] if j < 6 else kTb[:, j - 6, :]
                if j % 2 == 0:
                    nc.scalar.activation(out=dst, in_=ps[:], func=Identity,
                                         bias=qkb_sb[:, j:j + 1], scale=1.0)
                else:
                    nc.vector.tensor_scalar(out=dst, in0=ps[:],
                                            scalar1=qkb_sb[:, j:j + 1], scalar2=None,
                                            op0=add)
                yield

            # block-diagonal remainder tiles for the 4 leftover key tokens:
            # kTrem[:, cth, 0:4] = even head's k-remainder (d-rows 0:64),
            # kTrem[:, cth, 32:36] = odd head's (d-rows 64:128); other columns
            # zero so one matmul yields both heads' remainder scores.
            kTrem = qkpool.tile([128, 6, 36], bf16, tag="krem")
            st["krem"] = kTrem
            nc.vector.memset(kTrem[:], 0.0)
            nc.vector.tensor_copy(out=kTrem[0:64, :, 0:4], in_=kTb[0:64, :, 384:388])
            nc.vector.tensor_copy(out=kTrem[64:128, :, 32:36], in_=kTb[64:128, :, 384:388])

            vb = vpool.tile([128, 3, H, 65], bf16, tag="v")
            st["v"] = vb
            nc.vector.memset(vb[:, :, :, 64:65], 1.0)
            for ti, (t0, tp) in enumerate(TCH3 if "qkv" not in skip else []):
                for o0, on, hs, he in ((0, 512, 0, 8), (512, 256, 8, 12)):
                    pv = psum.tile([128, on], f32, tag="ps")
                    for ct in range(6):
                        nc.tensor.matmul(
                            pv[0:tp, 0:on],
                            lhsT=xTb[:, ct, t0:t0 + tp],
                            rhs=wT[:, ct, 1536 + o0:1536 + o0 + on],
                            start=(ct == 0), stop=(ct == 5),
                        )
                    nc.vector.tensor_tensor(
                        out=vb[0:tp, ti, hs:he, 0:64],
                        in0=pv[0:tp, :].rearrange("p (h d) -> p h d", h=he - hs),
                        in1=vb_bc[0:tp, o0:o0 + on].rearrange("p (h d) -> p h d", h=he - hs),
                        op=add,
                    )
                    yield

            # v-remainder regrouped to match kTrem's partition layout, from
            # the shared 8-batch remainder pass
            vrem = vpool.tile([36, 6, 65], bf16, tag="vrem")
            st["vrem"] = vrem
            if "qkv" not in skip:
                # DMA (no partition-alignment constraint) deinterleaves this
                # batch's remainder-v out of the shared 8-batch tile
                vr8 = rst["vr8"]
                nc.sync.dma_start(out=vrem[0:4, :, :],
                                  in_=vr8[b * 4:(b + 1) * 4, 0:12:2, :])
                nc.sync.dma_start(out=vrem[32:36, :, :],
                                  in_=vr8[b * 4:(b + 1) * 4, 1:12:2, :])
            else:
                nc.vector.memset(vrem[:], 0.0)
                nc.vector.memset(vb[:, :, :, 0:64], 0.0)

        def emit_attention(b, st, filler):
            """Attention heads; pulls filler items between scores and AVs.

            Softmax normalization is deferred and done per HEAD PAIR: the
            reciprocals of both heads' denominator rows (DVE, bf16) land in
            one [2,N] tile; a single [2,128]-mask PE matmul broadcasts them
            across the two 64-partition halves, ACT stages to SBUF, and two
            DVE multiplies normalize. This keeps gpsimd entirely out of the
            kernel and halves the broadcast/staging work vs per-head.
            """
            qTb, kTb, vb = st["q"], st["k"], st["v"]
            kTrem, vrem = st["krem"], st["vrem"]
            xattnT = apool.tile([128, 6, N], bf16, tag="xat")
            st["at"] = xattnT
            if "attn" in skip:
                for _ct in range(6):
                    nc.vector.tensor_copy(out=xattnT[:, _ct, :], in_=wT[:, 0, 0:N])
            nheads = H if "attn" not in skip else 0
            pulled = 0

            def flush_pair(pend):
                pav_e, pav_o, rinv2_p, cth_p = pend
                pbc = pbcpool.tile([128, N], f32, tag="pbc")
                nc.tensor.matmul(pbc[:], lhsT=mask33[:], rhs=rinv2_p[:],
                                 start=True, stop=True)
                # DVE can't read two PSUM operands; stage the broadcast in
                # SBUF via ACT (also keeps DVE free for the multiplies)
                rbf = spool.tile([128, N], f32, tag="rbf")
                nc.scalar.copy(out=rbf[:], in_=pbc[:])
                nc.vector.tensor_tensor(out=xattnT[0:64, cth_p, :],
                                        in0=pav_e[0:64, :], in1=rbf[0:64, :],
                                        op=mult)
                nc.vector.tensor_tensor(out=xattnT[64:128, cth_p, :],
                                        in0=pav_o[0:64, :], in1=rbf[64:128, :],
                                        op=mult)

            def emit_av(h, probs, prr, rinv2_t):
                """AV matmuls for head h (scores/exps were emitted one head
                earlier, so the exp outputs are ready — no PE stall)."""
                cth, r0 = h // 2, (h % 2) * 64
                pav = pavpool.tile([65, N], f32, tag="pav")
                nc.tensor.matmul(pav[:, 0:N], lhsT=vb[:, 0, h, :],
                                 rhs=probs[0][:, 0:N], start=True, stop=False)
                for kc, (t0, tp) in list(enumerate(TCH))[1:3]:
                    nc.tensor.matmul(pav[:, LT:N], lhsT=vb[0:tp, kc, h, :],
                                     rhs=probs[kc][0:tp, :],
                                     start=False, stop=False)
                rr = (h % 2) * 32
                nc.tensor.matmul(pav[:, LT:N], lhsT=vrem[rr:rr + 4, cth, :],
                                 rhs=prr[rr:rr + 4, :], start=False, stop=True)

                if "epi" in skip:
                    nc.vector.tensor_copy(out=xattnT[r0:r0 + 64, cth, :],
                                          in_=pav[0:64, :])
                    return None
                r32 = (h % 2) * 32
                with nc.allow_low_precision(reason="bf16 1/denom feeds a "
                                            "mask-broadcast matmul; ~0.2% "
                                            "rel err, budget is 2e-2"):
                    nc.vector.reciprocal(out=rinv2_t[r32:r32 + 1, :],
                                         in_=pav[64:65, :])
                return pav

            pend = None    # completed head pair awaiting normalize
            prev = None    # (h, probs, prr) awaiting AV for head h-1
            rinv2_t = None
            stash = None   # even head's pav awaiting its pair
            for h in range(nheads):
                cth, r0 = h // 2, (h % 2) * 64
                qh = qTb[r0:r0 + 64, cth, :]   # [64, 388] bf16
                kh = kTb[r0:r0 + 64, cth, :]

                # all scores matmuls first; chunk 0 covers ALL queries
                # (template cols 0:128 + search 128:388); the 4 remainder
                # keys are computed for the HEAD PAIR at even h via the
                # block-diagonal kTrem in one matmul + one exp
                probs = []
                for kc, (t0, tp) in list(enumerate(TCH))[0:3]:
                    pss = psum.tile([128, N if kc == 0 else LS], f32, tag="ps")
                    rhs_q = qh[:] if kc == 0 else qh[:, LT:N]
                    nc.tensor.matmul(pss[0:tp, :], lhsT=kh[:, t0:t0 + tp],
                                     rhs=rhs_q, start=True, stop=True)
                    prs = ppool.tile([128, N if kc == 0 else LS], bf16,
                                     tag="pr0" if kc == 0 else "prs")
                    nc.scalar.activation(out=prs[0:tp, :], in_=pss[0:tp, :],
                                         func=(Identity if "expid" in skip else Exp),
                                         scale=SCALE)
                    probs.append(prs)
                if h % 2 == 0:
                    psr = psum.tile([36, LS], f32, tag="ps")
                    nc.tensor.matmul(psr[:], lhsT=kTrem[:, cth, :],
                                     rhs=qTb[:, cth, LT:N], start=True, stop=True)
                    prr = ppool.tile([36, LS], bf16, tag="prr")
                    nc.scalar.activation(out=prr[:], in_=psr[:],
                                         func=(Identity if "expid" in skip else Exp),
                                         scale=SCALE)
                    st["prr"] = prr
                else:
                    prr = st["prr"]

                # filler work for neighbouring batches rides in the exp window
                want = (h + 1) * 22 // nheads
                while pulled < want and next(filler, "END") != "END":
                    pulled += 1

                # normalize the pair completed two heads ago, then AV for
                # head h-1: both consume results whose producers ran during
                # this head's scores
                if pend is not None and h % 2 == 1:
                    flush_pair(pend)
                    pend = None
                if prev is not None:
                    hp = prev[0]
                    if hp % 2 == 0:
                        rinv2_t = rinvA if (hp // 2) % 2 == 0 else rinvB
                        stash = emit_av(*prev, rinv2_t)
                    else:
                        pav_o = emit_av(*prev, rinv2_t)
                        if stash is not None and pav_o is not None:
                            pend = (stash, pav_o, rinv2_t, hp // 2)
                prev = (h, probs, prr)

            # tail: AV of the last head completes the final pair
            pend2 = None
            if prev is not None:
                hp = prev[0]
                pav_o = emit_av(*prev, rinv2_t)
                if stash is not None and pav_o is not None:
                    pend2 = (stash, pav_o, rinv2_t, hp // 2)
            if pend is not None:
                flush_pair(pend)
            # drain any remaining filler, then normalize the last pair
            while next(filler, "END") != "END":
                pass
            if "attn" not in skip and pend2 is not None:
                flush_pair(pend2)

        def proj_gen(b, st, rst, last):
            """Generator: 3 full proj chunk items; remainder tokens are
            staged into the rep-shared prem8 tile and projected for all 8
            batches at once after the rep's last batch."""
            if "proj" in skip:
                return
            xattnT = st["at"]
            for ti, (t0, tp) in enumerate(TCH3):
                osb = opool.tile([128, C], f32, tag="ob")
                for o0, on in ((0, 512), (512, 256)):
                    pp = psum.tile([128, on], f32, tag="ps")
                    for ct in range(6):
                        nc.tensor.matmul(
                            pp[0:tp, 0:on],
                            lhsT=xattnT[:, ct, t0:t0 + tp],
                            rhs=projT[:, ct, o0:o0 + on],
                            start=(ct == 0), stop=(ct == 5),
                        )
                    nc.vector.tensor_tensor(out=osb[0:tp, o0:o0 + on], in0=pp[0:tp, :],
                                            in1=pb_bc[0:tp, o0:o0 + on], op=add)
                nc.sync.dma_start(out=out_ext[b, t0:t0 + tp, :], in_=osb[0:tp, :])
                yield
            # stage this batch's remainder columns into the rep-shared tile
            prem8 = rst["prem8"]
            nc.vector.tensor_copy(out=prem8[:, :, b * 4:(b + 1) * 4],
                                  in_=xattnT[:, :, 384:388])
            yield
            if last:
                osb = opool.tile([32, C], f32, tag="obr")
                for o0, on in ((0, 512), (512, 256)):
                    pp = psum.tile([32, on], f32, tag="ps")
                    for ct in range(6):
                        nc.tensor.matmul(
                            pp[:, 0:on],
                            lhsT=prem8[:, ct, :],
                            rhs=projT[:, ct, o0:o0 + on],
                            start=(ct == 0), stop=(ct == 5),
                        )
                    nc.vector.tensor_tensor(out=osb[:, o0:o0 + on], in0=pp[:],
                                            in1=pb_bc[0:32, o0:o0 + on], op=add)
                for bb in range(BL):
                    nc.sync.dma_start(out=out_ext[bb, 384:388, :],
                                      in_=osb[bb * 4:(bb + 1) * 4, :])
                yield

        # ---- software-pipelined batch loop ----
        from itertools import chain

        seq = [bb for _ in range(reps) for bb in range(BL)]
        states = [dict() for _ in seq]
        rstates = [dict() for _ in range(reps)]

        # rep-0 remainder partA first: its tiny DMAs+cast+transposes complete
        # while the first big x-chunk DMA is still in flight, so the PE's
        # first real work is not delayed.
        for _ in vrem_partA(rstates[0]):
            pass
        gw = weights_gen()
        xf0 = emit_xload(seq[0])
        g0 = stage1(seq[0], xf0, states[0], rstates[0])
        gB = vrem_partB(rstates[0])
        # prologue: interleave the 24 weight-prep chunks with batch 0's
        # stage1 items (21: 3 transposes + 12 qk + 6 v). Transposes need no
        # weights; qk group j (item 3+j) needs weight chunk j; v items
        # (idx 15..20) need weight chunks 12-17. Give gw a 2-chunk lead
        # after the transposes so every consumer finds its weights already
        # emitted in PE program order (gw call m completes chunks 0..m-2).
        for i in range(3):
            next(g0, None)
            next(gw, None)
        next(gw, None)
        next(gw, None)
        for i in range(18):
            next(gw, None)
            next(g0, None)
        for _ in gw:
            pass
        # partB (produces vr8) must be emitted before g0's tail, which
        # copies this batch's vrem slices out of vr8.
        for _ in gB:
            pass
        for _ in g0:
            pass

        prev_proj = iter(())
        for i, b in enumerate(seq):
            r = i // BL
            extra = iter(())
            if i % BL == BL - 1 and r + 1 < reps:
                rstn = rstates[r + 1]
                extra = chain(vrem_partA(rstn), vrem_partB(rstn))
            if i + 1 < len(seq):
                xf_n = emit_xload(seq[i + 1])
                nxt = stage1(seq[i + 1], xf_n, states[i + 1],
                             rstates[(i + 1) // BL])
            else:
                nxt = iter(())
            emit_attention(b, states[i], chain(prev_proj, extra, nxt))
            prev_proj = proj_gen(b, states[i], rstates[r],
                                 last=(i % BL == BL - 1))
        for _ in prev_proj:
            pass

    nc.compile()
    return nc


def _get_nc():
    if "nc" not in _NC_CACHE:
        _NC_CACHE["nc"] = _build_nc()
    return _NC_CACHE["nc"]


def kernel(x, qkv_w, qkv_b, proj_w, proj_b, t_h=8, t_w=8, s_h=16, s_w=16):
    from concourse.bass_utils import run_bass_kernel_spmd

    x = np.ascontiguousarray(np.asarray(x, dtype=np.float32))
    qkv_w = np.ascontiguousarray(np.asarray(qkv_w, dtype=np.float32))
    qkv_b = np.ascontiguousarray(np.asarray(qkv_b, dtype=np.float32))
    proj_w = np.ascontiguousarray(np.asarray(proj_w, dtype=np.float32))
    proj_b = np.ascontiguousarray(np.asarray(proj_b, dtype=np.float32))

    nc = _get_nc()
    in_maps = [
        {
            "x": x[i * BL:(i + 1) * BL],
            "qkv_w": qkv_w,
            "qkv_b": qkv_b,
            "proj_w": proj_w,
            "proj_b": proj_b,
        }
        for i in range(NCORES)
    ]
    res = run_bass_kernel_spmd(nc, in_maps, core_ids=list(range(NCORES)))
    out = np.concatenate([res.results[i]["out"] for i in range(NCORES)], axis=0)
    return out.astype(np.float32)


# revision 33
# speedup vs baseline: 1.1936x; 1.1936x over previous
"""Sparse-attention Trainium2 kernel, 8-way data-parallel over batch.

Reference computation (per batch):
  qkv = x @ qkv_w.T + qkv_b              -> split q,k,v [H=12, N=388, D=64]
  template queries (tokens 0:128) attend to template keys (0:128)
  search queries (tokens 128:388) attend to all 388 keys
  out = concat @ proj_w.T + proj_b

Kernel strategy per core (B_local=8 batches, all compute on device, bf16
matmuls with fp32 PSUM accumulation):
  - x cast to bf16, transposed feature-major via PE transposes.
  - q^T,k^T = W^T-stationary matmuls (feature-major out, per-partition bias
    added in fp32 on ACT/DVE during the PSUM->SBUF copy).
  - v = x^T-stationary matmuls (token-major out), stored per-head with a ones
    column appended so the attention-value matmul also produces softmax sums.
  - ALL remainder-token work (tokens 384:388 of each batch) is hoisted into
    shared 8-batch passes: one upfront pass builds the remainder x^T columns
    and remainder v for all 8 batches; one final pass projects all 8 batches'
    remainder tokens.  This removes the tiny per-batch matmuls whose
    dispatch+ldweights cost dominates their streaming time.
  - scores computed TRANSPOSED: S^T[k,q] = k^T-slices as lhsT, q^T as rhs.
    exp on ACT (scale=1/8 folded in), probs in bf16.
  - AV: out^T[d,q] accumulated over k-chunks; row 64 = softmax denominators.
    AV for head h is emitted during head h+1's scores so the exps are ready.
  - normalize (deferred, per HEAD PAIR): reciprocals (DVE, bf16) of both
    heads' denominator rows go into one [2,N] tile; a single [2,128] 0/1-mask
    PE matmul broadcasts head-even's recip to partitions 0:64 and head-odd's
    to 64:128, ACT-staged to SBUF, then two DVE multiplies normalize.
  - proj matmul reads attention output directly (no transposes), bias on DVE,
    DMA out token-major fp32.
  - weight prep: fp32 DMA, DVE cast to bf16 (prefetched 2 chunks ahead),
    1-cycle/row PE transposes.
"""

import numpy as np

B, N, C = 64, 388, 768
H, D = 12, 64
LT = 128          # template tokens (= first token chunk, exactly)
LS = N - LT       # 260 search tokens
NCORES = 8
BL = B // NCORES  # 8 batches per core
O3 = 3 * C        # 2304
SCALE = 0.125

_NC_CACHE = {}


def _build_nc(dump=False, reps=1, skip=()):
    from contextlib import ExitStack

    import concourse.tile as tile
    from concourse import bacc, mybir
    from concourse.masks import make_identity

    f32 = mybir.dt.float32
    bf16 = mybir.dt.bfloat16
    Identity = mybir.ActivationFunctionType.Identity
    Exp = mybir.ActivationFunctionType.Exp
    mult = mybir.AluOpType.mult
    add = mybir.AluOpType.add

    nc = bacc.Bacc("TRN2", target_bir_lowering=False)

    x_ext = nc.dram_tensor("x", [BL, N, C], f32, kind="ExternalInput")
    qkvw_ext = nc.dram_tensor("qkv_w", [O3, C], f32, kind="ExternalInput")
    qkvb_ext = nc.dram_tensor("qkv_b", [O3], f32, kind="ExternalInput")
    projw_ext = nc.dram_tensor("proj_w", [C, C], f32, kind="ExternalInput")
    projb_ext = nc.dram_tensor("proj_b", [C], f32, kind="ExternalInput")
    out_ext = nc.dram_tensor("out", [BL, N, C], f32, kind="ExternalOutput")

    # token chunking of the 388 tokens: 128,128,128 + 4 remainder (hoisted)
    TCH = [(0, 128), (128, 128), (256, 128), (384, 4)]
    TCH3 = TCH[0:3]

    with tile.TileContext(nc) as tc, ExitStack() as ctx:
        const = ctx.enter_context(tc.tile_pool(name="const", bufs=1))
        stage = ctx.enter_context(tc.tile_pool(name="stage", bufs=4))
        # per-rep shared tiles (remainder-token passes)
        rpool = ctx.enter_context(tc.tile_pool(name="rp", bufs=2))
        # 8 PSUM banks total: 5 general + 2 deferred-AV accumulators + 1
        # reciprocal-broadcast target
        psum = ctx.enter_context(tc.tile_pool(name="ps", bufs=5, space="PSUM"))
        pavpool = ctx.enter_context(tc.tile_pool(name="pav", bufs=2, space="PSUM"))
        pbcpool = ctx.enter_context(tc.tile_pool(name="pbc", bufs=1, space="PSUM"))

        ident = const.tile([128, 128], f32)
        make_identity(nc, ident)
        ident_b = const.tile([128, 128], bf16)
        make_identity(nc, ident_b)
        # head-pair normalize broadcast mask: row0 -> partitions 0:64,
        # row32 -> partitions 64:128 (engine writes must start at partition
        # 0/32/64/96, so the two reciprocals land on rows 0 and 32 of a
        # [33,N] tile; mask rows 1..31 are zero so those rows don't
        # contribute). Two ping-pong rinv tiles (memset once) avoid a
        # per-pair clear of the unused rows.
        mask33 = const.tile([33, 128], bf16)
        nc.vector.memset(mask33[:], 0.0)
        nc.vector.memset(mask33[0:1, 0:64], 1.0)
        nc.vector.memset(mask33[32:33, 64:128], 1.0)
        rinvA = const.tile([33, N], bf16)
        rinvB = const.tile([33, N], bf16)
        nc.vector.memset(rinvA[0:32, :], 0.0)
        nc.vector.memset(rinvB[0:32, :], 0.0)

        # ---- weights/biases declared here; emission interleaved with batch 0
        wT = const.tile([128, 6, O3], bf16)
        projT = const.tile([128, 6, C], bf16)
        qkb_sb = const.tile([128, 12], f32)
        vb_bc = const.tile([128, C], f32)
        pb_bc = const.tile([128, C], f32)

        def weights_gen():
            qb_st = stage.tile([12, 128], f32, tag="bst")
            nc.sync.dma_start(out=qb_st[:], in_=qkvb_ext[0:1536].rearrange("(j p) -> j p", p=128))
            pbt = psum.tile([128, 12], f32, tag="ps")
            nc.tensor.transpose(pbt[:], qb_st[:], ident[0:12, 0:12])
            nc.scalar.copy(out=qkb_sb[:], in_=pbt[:])

            # weight chunks: DMA fp32, cast bf16 on DVE, then 1-cycle/row PE
            # transposes. DMA+cast run two chunks ahead so the PE never waits
            # on the DMA->cast latency chain.
            wstbs = {}

            def fetch(j):
                wstf = stage.tile([128, C], f32, tag="wstf")
                src = qkvw_ext[j * 128:(j + 1) * 128, :] if j < 18 else \
                    projw_ext[(j - 18) * 128:(j - 17) * 128, :]
                nc.sync.dma_start(out=wstf[:], in_=src)
                wstb = stage.tile([128, C], bf16, tag="wstb")
                nc.vector.tensor_copy(out=wstb[:], in_=wstf[:])
                wstbs[j] = wstb

            fetch(0)
            fetch(1)
            for j in range(24):
                if j == 4:
                    nc.sync.dma_start(out=vb_bc[:], in_=qkvb_ext[1536:2304].unsqueeze(0).to_broadcast([128, C]))
                    nc.sync.dma_start(out=pb_bc[:], in_=projb_ext[:].unsqueeze(0).to_broadcast([128, C]))
                if j + 2 < 24:
                    fetch(j + 2)
                wstb = wstbs.pop(j)
                wdst = wT if j < 18 else projT
                jo = j * 128 if j < 18 else (j - 18) * 128
                # all 6 transposes into one bf16 PSUM bank, single copy out
                pt = psum.tile([128, 6 * 128], bf16, tag="ps")
                for ct in range(6):
                    nc.tensor.transpose(pt[:, ct * 128:(ct + 1) * 128],
                                        wstb[:, ct * 128:(ct + 1) * 128],
                                        ident_b[:])
                dst = wdst[:, :, jo:jo + 128]
                src_ap = pt[:].rearrange("p (g t) -> p g t", g=6)
                if j % 2 == 0:
                    nc.scalar.copy(out=dst, in_=src_ap)
                else:
                    nc.vector.tensor_copy(out=dst, in_=src_ap)
                yield

        # ---- per-rep remainder-token passes --------------------------------
        def vrem_partA(rst):
            """Remainder x rows for all 8 batches -> feature-major xT8
            [128, 6, 32] (32 = batch-major remainder tokens)."""
            xrem_f = rpool.tile([32, C], f32, tag="xrf")
            for b in range(BL):
                nc.sync.dma_start(out=xrem_f[b * 4:(b + 1) * 4, :],
                                  in_=x_ext[b, 384:388, :])
            xrem_c = rpool.tile([32, C], bf16, tag="xrc")
            nc.scalar.copy(out=xrem_c[:], in_=xrem_f[:])
            xT8 = rpool.tile([128, 6, 32], bf16, tag="xt8")
            rst["xT8"] = xT8
            pt = psum.tile([128, 192], bf16, tag="ps")
            for ct in range(6):
                nc.tensor.transpose(pt[:, ct * 32:(ct + 1) * 32],
                                    xrem_c[:, ct * 128:(ct + 1) * 128],
                                    ident_b[0:32, 0:32])
            nc.vector.tensor_copy(out=xT8[:],
                                  in_=pt[:].rearrange("p (g t) -> p g t", g=6))
            # allocate the rep's shared tiles here so stage1/proj_gen can
            # reference them before partB's matmuls are emitted
            vr8 = rpool.tile([32, H, 65], bf16, tag="vr8")
            prem8 = rpool.tile([128, 6, 32], bf16, tag="pr8")
            rst["vr8"] = vr8
            rst["prem8"] = prem8
            yield

        def vrem_partB(rst):
            """Remainder v (+ones col) for all 8 batches: vr8 [32, H, 65]."""
            xT8 = rst["xT8"]
            vr8 = rst["vr8"]
            nc.vector.memset(vr8[:, :, 64:65], 1.0)
            if "qkv" in skip:
                nc.vector.memset(vr8[:, :, 0:64], 0.0)
                return
            for o0, on, hs, he in ((0, 512, 0, 8), (512, 256, 8, 12)):
                pv = psum.tile([32, on], f32, tag="ps")
                for ct in range(6):
                    nc.tensor.matmul(
                        pv[:, 0:on],
                        lhsT=xT8[:, ct, :],
                        rhs=wT[:, ct, 1536 + o0:1536 + o0 + on],
                        start=(ct == 0), stop=(ct == 5),
                    )
                nc.vector.tensor_tensor(
                    out=vr8[:, hs:he, 0:64],
                    in0=pv[:].rearrange("p (h d) -> p h d", h=he - hs),
                    in1=vb_bc[0:32, o0:o0 + on].rearrange("p (h d) -> p h d", h=he - hs),
                    op=add,
                )
                yield

        # ---- per-batch pools ----
        xpool = ctx.enter_context(tc.tile_pool(name="xp", bufs=2))
        xtpool = ctx.enter_context(tc.tile_pool(name="xtp", bufs=2))
        qkpool = ctx.enter_context(tc.tile_pool(name="qkp", bufs=2))
        vpool = ctx.enter_context(tc.tile_pool(name="vp", bufs=2))
        apool = ctx.enter_context(tc.tile_pool(name="ap", bufs=2))
        ppool = ctx.enter_context(tc.tile_pool(name="pp", bufs=4))
        spool = ctx.enter_context(tc.tile_pool(name="ssp", bufs=4))
        opool = ctx.enter_context(tc.tile_pool(name="op", bufs=3))

        def emit_xload(b):
            # one DMA per 128-token chunk: the first transpose only waits for
            # chunk 0 (~1/3 of the full-x DMA time). Remainder rows 384:388
            # are handled by the shared vrem pass.
            xf = xpool.tile([128, 3, C], f32, tag="xf")
            for ti in range(3):
                nc.sync.dma_start(out=xf[:, ti, :],
                                  in_=x_ext[b, ti * 128:(ti + 1) * 128, :])
            return xf

        def stage1(b, xf, st, rst):
            """Generator: transposes (3 items), q/k groups (12), v (6).
            Yields between PE-work units so attention of the previous batch
            can interleave. Fills `st` with the batch's tiles."""
            xTb = xtpool.tile([128, 6, N], bf16, tag="xt")
            st["xT"] = xTb
            xc = xpool.tile([128, 3, C], bf16, tag="xc")
            for ti, (t0, tp) in enumerate(TCH3):
                # cast this chunk to bf16 on ACT, then 1-cycle/row transposes
                # (all 6 into one bf16 PSUM bank, single copy out)
                nc.scalar.copy(out=xc[0:tp, ti, :], in_=xf[0:tp, ti, :])
                pt = psum.tile([128, 6 * tp], bf16, tag="ps")
                for ct in range(6):
                    nc.tensor.transpose(pt[:, ct * tp:(ct + 1) * tp],
                                        xc[0:tp, ti, ct * 128:(ct + 1) * 128],
                                        ident_b[0:tp, 0:tp])
                dst = xTb[:, :, t0:t0 + tp]
                src_ap = pt[:].rearrange("p (g t) -> p g t", g=6)
                if ti % 2 == 0:
                    nc.vector.tensor_copy(out=dst, in_=src_ap)
                else:
                    nc.scalar.copy(out=dst, in_=src_ap)
                yield
            # remainder columns from the shared 8-batch pass
            nc.vector.tensor_copy(out=xTb[:, :, 384:388],
                                  in_=rst["xT8"][:, :, b * 4:(b + 1) * 4])

            qTb = qkpool.tile([128, 6, N], bf16, tag="q")
            kTb = qkpool.tile([128, 6, N], bf16, tag="k")
            st["q"], st["k"] = qTb, kTb
            if "qkv" in skip:
                nc.vector.memset(qTb[:, 0:1, 0:2], 0.0)
                nc.vector.memset(kTb[:, 0:1, 0:2], 0.0)
            for j in range(12 if "qkv" not in skip else 0):
                ps = psum.tile([128, N], f32, tag="ps")
                for ct in range(6):
                    nc.tensor.matmul(
                        ps[:],
                        lhsT=wT[:, ct, j * 128:(j + 1) * 128],
                        rhs=xTb[:, ct, :],
                        start=(ct == 0), stop=(ct == 5),
                    )
                dst = qTb[:, j, :] if j < 6 else kTb[:, j - 6, :]
                if j % 2 == 0:
                    nc.scalar.activation(out=dst, in_=ps[:], func=Identity,
                                         bias=qkb_sb[:, j:j + 1], scale=1.0)
                else:
                    nc.vector.tensor_scalar(out=dst, in0=ps[:],
                                            scalar1=qkb_sb[:, j:j + 1], scalar2=None,
                                            op0=add)
                yield

            # block-diagonal remainder tiles for the 4 leftover key tokens:
            # kTrem[:, cth, 0:4] = even head's k-remainder (d-rows 0:64),
            # kTrem[:, cth, 32:36] = odd head's (d-rows 64:128); other columns
            # zero so one matmul yields both heads' remainder scores.
            kTrem = qkpool.tile([128, 6, 36], bf16, tag="krem")
            st["krem"] = kTrem
            nc.vector.memset(kTrem[:], 0.0)
            nc.vector.tensor_copy(out=kTrem[0:64, :, 0:4], in_=kTb[0:64, :, 384:388])
            nc.vector.tensor_copy(out=kTrem[64:128, :, 32:36], in_=kTb[64:128, :, 384:388])

            vb = vpool.tile([128, 3, H, 65], bf16, tag="v")
            st["v"] = vb
            nc.vector.memset(vb[:, :, :, 64:65], 1.0)
            for ti, (t0, tp) in enumerate(TCH3 if "qkv" not in skip else []):
                for o0, on, hs, he in ((0, 512, 0, 8), (512, 256, 8, 12)):
                    pv = psum.tile([128, on], f32, tag="ps")
                    for ct in range(6):
                        nc.tensor.matmul(
                            pv[0:tp, 0:on],
                            lhsT=xTb[:, ct, t0:t0 + tp],
                            rhs=wT[:, ct, 1536 + o0:1536 + o0 + on],
                            start=(ct == 0), stop=(ct == 5),
                        )
                    nc.vector.tensor_tensor(
                        out=vb[0:tp, ti, hs:he, 0:64],
                        in0=pv[0:tp, :].rearrange("p (h d) -> p h d", h=he - hs),
                        in1=vb_bc[0:tp, o0:o0 + on].rearrange("p (h d) -> p h d", h=he - hs),
                        op=add,
                    )
                    yield

            # v-remainder regrouped to match kTrem's partition layout, from
            # the shared 8-batch remainder pass
            vrem = vpool.tile([36, 6, 65], bf16, tag="vrem")
            st["vrem"] = vrem
            if "qkv" not in skip:
                # DMA (no partition-alignment constraint) deinterleaves this
                # batch's remainder-v out of the shared 8-batch tile
                vr8 = rst["vr8"]
                nc.sync.dma_start(out=vrem[0:4, :, :],
                                  in_=vr8[b * 4:(b + 1) * 4, 0:12:2, :])
                nc.sync.dma_start(out=vrem[32:36, :, :],
                                  in_=vr8[b * 4:(b + 1) * 4, 1:12:2, :])
            else:
                nc.vector.memset(vrem[:], 0.0)
                nc.vector.memset(vb[:, :, :, 0:64], 0.0)

        def emit_attention(b, st, filler):
            """Attention heads; pulls filler items between scores and AVs.

            Softmax normalization is deferred and done per HEAD PAIR: the
            reciprocals of both heads' denominator rows (DVE, bf16) land in
            one [2,N] tile; a single [2,128]-mask PE matmul broadcasts them
            across the two 64-partition halves, ACT stages to SBUF, and two
            DVE multiplies normalize. This keeps gpsimd entirely out of the
            kernel and halves the broadcast/staging work vs per-head.
            """
            qTb, kTb, vb = st["q"], st["k"], st["v"]
            kTrem, vrem = st["krem"], st["vrem"]
            xattnT = apool.tile([128, 6, N], bf16, tag="xat")
            st["at"] = xattnT
            if "attn" in skip:
                for _ct in range(6):
                    nc.vector.tensor_copy(out=xattnT[:, _ct, :], in_=wT[:, 0, 0:N])
            nheads = H if "attn" not in skip else 0
            pulled = 0

            def flush_pair(pend):
                pav_e, pav_o, rinv2_p, cth_p = pend
                pbc = pbcpool.tile([128, N], f32, tag="pbc")
                nc.tensor.matmul(pbc[:], lhsT=mask33[:], rhs=rinv2_p[:],
                                 start=True, stop=True)
                # DVE can't read two PSUM operands; stage the broadcast in
                # SBUF via ACT (also keeps DVE free for the multiplies)
                rbf = spool.tile([128, N], f32, tag="rbf")
                nc.scalar.copy(out=rbf[:], in_=pbc[:])
                nc.vector.tensor_tensor(out=xattnT[0:64, cth_p, :],
                                        in0=pav_e[0:64, :], in1=rbf[0:64, :],
                                        op=mult)
                nc.vector.tensor_tensor(out=xattnT[64:128, cth_p, :],
                                        in0=pav_o[0:64, :], in1=rbf[64:128, :],
                                        op=mult)

            def emit_av(h, probs, prr, rinv2_t):
                """AV matmuls for head h (scores/exps were emitted one head
                earlier, so the exp outputs are ready — no PE stall)."""
                cth, r0 = h // 2, (h % 2) * 64
                pav = pavpool.tile([65, N], f32, tag="pav")
                nc.tensor.matmul(pav[:, 0:N], lhsT=vb[:, 0, h, :],
                                 rhs=probs[0][:, 0:N], start=True, stop=False)
                for kc, (t0, tp) in list(enumerate(TCH))[1:3]:
                    nc.tensor.matmul(pav[:, LT:N], lhsT=vb[0:tp, kc, h, :],
                                     rhs=probs[kc][0:tp, :],
                                     start=False, stop=False)
                rr = (h % 2) * 32
                nc.tensor.matmul(pav[:, LT:N], lhsT=vrem[rr:rr + 4, cth, :],
                                 rhs=prr[rr:rr + 4, :], start=False, stop=True)

                if "epi" in skip:
                    nc.vector.tensor_copy(out=xattnT[r0:r0 + 64, cth, :],
                                          in_=pav[0:64, :])
                    return None
                return pav

            def emit_recips(pend):
                """Both reciprocals of a completed pair, emitted at the TOP
                of the flush iteration (just-in-time): their AV-stop waits
                are already satisfied, so they don't block the DVE queue
                head for two head-iterations the way emitting them inside
                emit_av did."""
                pav_e, pav_o, rinv2_p, cth_p = pend
                with nc.allow_low_precision(reason="bf16 1/denom feeds a "
                                            "mask-broadcast matmul; ~0.2% "
                                            "rel err, budget is 2e-2"):
                    nc.vector.reciprocal(out=rinv2_p[0:1, :],
                                         in_=pav_e[64:65, :])
                    nc.vector.reciprocal(out=rinv2_p[32:33, :],
                                         in_=pav_o[64:65, :])

            pend = None    # completed head pair awaiting normalize
            prev = None    # (h, probs, prr) awaiting AV for head h-1
            rinv2_t = None
            stash = None   # even head's pav awaiting its pair
            for h in range(nheads):
                cth, r0 = h // 2, (h % 2) * 64
                qh = qTb[r0:r0 + 64, cth, :]   # [64, 388] bf16
                kh = kTb[r0:r0 + 64, cth, :]

                # reciprocals of the pair to be flushed this iteration run
                # on DVE under the scores/filler window below
                if pend is not None and h % 2 == 1:
                    emit_recips(pend)

                # all scores matmuls first; chunk 0 covers ALL queries
                # (template cols 0:128 + search 128:388); the 4 remainder
                # keys are computed for the HEAD PAIR at even h via the
                # block-diagonal kTrem in one matmul + one exp
                probs = []
                for kc, (t0, tp) in list(enumerate(TCH))[0:3]:
                    pss = psum.tile([128, N if kc == 0 else LS], f32, tag="ps")
                    rhs_q = qh[:] if kc == 0 else qh[:, LT:N]
                    nc.tensor.matmul(pss[0:tp, :], lhsT=kh[:, t0:t0 + tp],
                                     rhs=rhs_q, start=True, stop=True)
                    prs = ppool.tile([128, N if kc == 0 else LS], bf16,
                                     tag="pr0" if kc == 0 else "prs")
                    nc.scalar.activation(out=prs[0:tp, :], in_=pss[0:tp, :],
                                         func=(Identity if "expid" in skip else Exp),
                                         scale=SCALE)
                    probs.append(prs)
                if h % 2 == 0:
                    psr = psum.tile([36, LS], f32, tag="ps")
                    nc.tensor.matmul(psr[:], lhsT=kTrem[:, cth, :],
                                     rhs=qTb[:, cth, LT:N], start=True, stop=True)
                    prr = ppool.tile([36, LS], bf16, tag="prr")
                    nc.scalar.activation(out=prr[:], in_=psr[:],
                                         func=(Identity if "expid" in skip else Exp),
                                         scale=SCALE)
                    st["prr"] = prr
                else:
                    prr = st["prr"]

                # filler work for neighbouring batches rides in the exp window
                want = (h + 1) * 22 // nheads
                while pulled < want and next(filler, "END") != "END":
                    pulled += 1

                # normalize the pair completed two heads ago, then AV for
                # head h-1: both consume results whose producers ran during
                # this head's scores
                if pend is not None and h % 2 == 1:
                    flush_pair(pend)
                    pend = None
                if prev is not None:
                    hp = prev[0]
                    if hp % 2 == 0:
                        rinv2_t = rinvA if (hp // 2) % 2 == 0 else rinvB
                        stash = emit_av(*prev, rinv2_t)
                    else:
                        pav_o = emit_av(*prev, rinv2_t)
                        if stash is not None and pav_o is not None:
                            pend = (stash, pav_o, rinv2_t, hp // 2)
                prev = (h, probs, prr)

            # tail: AV of the last head completes the final pair
            pend2 = None
            if prev is not None:
                hp = prev[0]
                pav_o = emit_av(*prev, rinv2_t)
                if stash is not None and pav_o is not None:
                    pend2 = (stash, pav_o, rinv2_t, hp // 2)
            if pend is not None:
                emit_recips(pend)
                flush_pair(pend)
            if pend2 is not None:
                emit_recips(pend2)
            # drain any remaining filler, then normalize the last pair
            while next(filler, "END") != "END":
                pass
            if "attn" not in skip and pend2 is not None:
                flush_pair(pend2)

        def proj_gen(b, st, rst, last):
            """Generator: 3 full proj chunk items; remainder tokens are
            staged into the rep-shared prem8 tile and projected for all 8
            batches at once after the rep's last batch."""
            if "proj" in skip:
                return
            xattnT = st["at"]
            for ti, (t0, tp) in enumerate(TCH3):
                osb = opool.tile([128, C], f32, tag="ob")
                for o0, on in ((0, 512), (512, 256)):
                    pp = psum.tile([128, on], f32, tag="ps")
                    for ct in range(6):
                        nc.tensor.matmul(
                            pp[0:tp, 0:on],
                            lhsT=xattnT[:, ct, t0:t0 + tp],
                            rhs=projT[:, ct, o0:o0 + on],
                            start=(ct == 0), stop=(ct == 5),
                        )
                    nc.vector.tensor_tensor(out=osb[0:tp, o0:o0 + on], in0=pp[0:tp, :],
                                            in1=pb_bc[0:tp, o0:o0 + on], op=add)
                nc.sync.dma_start(out=out_ext[b, t0:t0 + tp, :], in_=osb[0:tp, :])
                yield
            # stage this batch's remainder columns into the rep-shared tile
            prem8 = rst["prem8"]
            nc.vector.tensor_copy(out=prem8[:, :, b * 4:(b + 1) * 4],
                                  in_=xattnT[:, :, 384:388])
            yield
            if last:
                osb = opool.tile([32, C], f32, tag="obr")
                for o0, on in ((0, 512), (512, 256)):
                    pp = psum.tile([32, on], f32, tag="ps")
                    for ct in range(6):
                        nc.tensor.matmul(
                            pp[:, 0:on],
                            lhsT=prem8[:, ct, :],
                            rhs=projT[:, ct, o0:o0 + on],
                            start=(ct == 0), stop=(ct == 5),
                        )
                    nc.vector.tensor_tensor(out=osb[:, o0:o0 + on], in0=pp[:],
                                            in1=pb_bc[0:32, o0:o0 + on], op=add)
                for bb in range(BL):
                    nc.sync.dma_start(out=out_ext[bb, 384:388, :],
                                      in_=osb[bb * 4:(bb + 1) * 4, :])
                yield

        # ---- software-pipelined batch loop ----
        from itertools import chain

        seq = [bb for _ in range(reps) for bb in range(BL)]
        states = [dict() for _ in seq]
        rstates = [dict() for _ in range(reps)]

        # rep-0 remainder partA first: its tiny DMAs+cast+transposes complete
        # while the first big x-chunk DMA is still in flight, so the PE's
        # first real work is not delayed.
        for _ in vrem_partA(rstates[0]):
            pass
        gw = weights_gen()
        xf0 = emit_xload(seq[0])
        g0 = stage1(seq[0], xf0, states[0], rstates[0])
        gB = vrem_partB(rstates[0])
        # prologue: interleave the 24 weight-prep chunks with batch 0's
        # stage1 items (21: 3 transposes + 12 qk + 6 v). Transposes need no
        # weights; qk group j (item 3+j) needs weight chunk j; v items
        # (idx 15..20) need weight chunks 12-17. Give gw a 2-chunk lead
        # after the transposes so every consumer finds its weights already
        # emitted in PE program order (gw call m completes chunks 0..m-2).
        for i in range(3):
            next(g0, None)
            next(gw, None)
        next(gw, None)
        next(gw, None)
        for i in range(18):
            next(gw, None)
            next(g0, None)
        for _ in gw:
            pass
        # partB (produces vr8) must be emitted before g0's tail, which
        # copies this batch's vrem slices out of vr8.
        for _ in gB:
            pass
        for _ in g0:
            pass

        prev_proj = iter(())
        for i, b in enumerate(seq):
            r = i // BL
            extra = iter(())
            if i % BL == BL - 1 and r + 1 < reps:
                rstn = rstates[r + 1]
                extra = chain(vrem_partA(rstn), vrem_partB(rstn))
            if i + 1 < len(seq):
                xf_n = emit_xload(seq[i + 1])
                nxt = stage1(seq[i + 1], xf_n, states[i + 1],
                             rstates[(i + 1) // BL])
            else:
                nxt = iter(())
            emit_attention(b, states[i], chain(prev_proj, extra, nxt))
            prev_proj = proj_gen(b, states[i], rstates[r],
                                 last=(i % BL == BL - 1))
        for _ in prev_proj:
            pass

    nc.compile()
    return nc


def _get_nc():
    if "nc" not in _NC_CACHE:
        _NC_CACHE["nc"] = _build_nc()
    return _NC_CACHE["nc"]


def kernel(x, qkv_w, qkv_b, proj_w, proj_b, t_h=8, t_w=8, s_h=16, s_w=16):
    from concourse.bass_utils import run_bass_kernel_spmd

    x = np.ascontiguousarray(np.asarray(x, dtype=np.float32))
    qkv_w = np.ascontiguousarray(np.asarray(qkv_w, dtype=np.float32))
    qkv_b = np.ascontiguousarray(np.asarray(qkv_b, dtype=np.float32))
    proj_w = np.ascontiguousarray(np.asarray(proj_w, dtype=np.float32))
    proj_b = np.ascontiguousarray(np.asarray(proj_b, dtype=np.float32))

    nc = _get_nc()
    in_maps = [
        {
            "x": x[i * BL:(i + 1) * BL],
            "qkv_w": qkv_w,
            "qkv_b": qkv_b,
            "proj_w": proj_w,
            "proj_b": proj_b,
        }
        for i in range(NCORES)
    ]
    res = run_bass_kernel_spmd(nc, in_maps, core_ids=list(range(NCORES)))
    out = np.concatenate([res.results[i]["out"] for i in range(NCORES)], axis=0)
    return out.astype(np.float32)


# revision 40
# speedup vs baseline: 1.2123x; 1.0156x over previous
"""Sparse-attention Trainium2 kernel, 8-way data-parallel over batch.

Reference computation (per batch):
  qkv = x @ qkv_w.T + qkv_b              -> split q,k,v [H=12, N=388, D=64]
  template queries (tokens 0:128) attend to template keys (0:128)
  search queries (tokens 128:388) attend to all 388 keys
  out = concat @ proj_w.T + proj_b

Kernel strategy per core (B_local=8 batches, all compute on device, bf16
matmuls with fp32 PSUM accumulation):
  - x cast to bf16, transposed feature-major via PE transposes.
  - q^T,k^T = W^T-stationary matmuls (feature-major out, per-partition bias
    added in fp32 on ACT/DVE during the PSUM->SBUF copy).
  - v = x^T-stationary matmuls (token-major out), stored per-head with a ones
    column appended so the attention-value matmul also produces softmax sums.
  - ALL remainder-token work (tokens 384:388 of each batch) is hoisted into
    shared 8-batch passes: one upfront pass builds the remainder x^T columns
    and remainder v for all 8 batches; one final pass projects all 8 batches'
    remainder tokens.  This removes the tiny per-batch matmuls whose
    dispatch+ldweights cost dominates their streaming time.
  - scores computed TRANSPOSED: S^T[k,q] = k^T-slices as lhsT, q^T as rhs.
    exp on ACT (scale=1/8 folded in), probs in bf16.
  - AV: out^T[d,q] accumulated over k-chunks; row 64 = softmax denominators.
    AV for head h is emitted during head h+1's scores so the exps are ready.
  - normalize (deferred, per HEAD PAIR): reciprocals (DVE, bf16) of both
    heads' denominator rows go into one [2,N] tile; a single [2,128] 0/1-mask
    PE matmul broadcasts head-even's recip to partitions 0:64 and head-odd's
    to 64:128, ACT-staged to SBUF, then two DVE multiplies normalize.
  - proj matmul reads attention output directly (no transposes), bias on DVE,
    DMA out token-major fp32.
  - weight prep: fp32 DMA, DVE cast to bf16 (prefetched 2 chunks ahead),
    1-cycle/row PE transposes.
"""

import numpy as np

B, N, C = 64, 388, 768
H, D = 12, 64
LT = 128          # template tokens (= first token chunk, exactly)
LS = N - LT       # 260 search tokens
NCORES = 8
BL = B // NCORES  # 8 batches per core
O3 = 3 * C        # 2304
SCALE = 0.125

_NC_CACHE = {}


def _build_nc(dump=False, reps=1, skip=()):
    from contextlib import ExitStack

    import concourse.tile as tile
    from concourse import bacc, mybir
    from concourse.masks import make_identity

    f32 = mybir.dt.float32
    bf16 = mybir.dt.bfloat16
    Identity = mybir.ActivationFunctionType.Identity
    Exp = mybir.ActivationFunctionType.Exp
    mult = mybir.AluOpType.mult
    add = mybir.AluOpType.add

    nc = bacc.Bacc("TRN2", target_bir_lowering=False)

    x_ext = nc.dram_tensor("x", [BL, N, C], f32, kind="ExternalInput")
    qkvw_ext = nc.dram_tensor("qkv_w", [O3, C], f32, kind="ExternalInput")
    qkvb_ext = nc.dram_tensor("qkv_b", [O3], f32, kind="ExternalInput")
    projw_ext = nc.dram_tensor("proj_w", [C, C], f32, kind="ExternalInput")
    projb_ext = nc.dram_tensor("proj_b", [C], f32, kind="ExternalInput")
    out_ext = nc.dram_tensor("out", [BL, N, C], f32, kind="ExternalOutput")

    # token chunking of the 388 tokens: 128,128,128 + 4 remainder (hoisted)
    TCH = [(0, 128), (128, 128), (256, 128), (384, 4)]
    TCH3 = TCH[0:3]

    with tile.TileContext(nc) as tc, ExitStack() as ctx:
        const = ctx.enter_context(tc.tile_pool(name="const", bufs=1))
        stage = ctx.enter_context(tc.tile_pool(name="stage", bufs=4))
        # per-rep shared tiles (remainder-token passes)
        rpool = ctx.enter_context(tc.tile_pool(name="rp", bufs=2))
        # 8 PSUM banks total: 5 general + 2 deferred-AV accumulators + 1
        # reciprocal-broadcast target
        psum = ctx.enter_context(tc.tile_pool(name="ps", bufs=5, space="PSUM"))
        pavpool = ctx.enter_context(tc.tile_pool(name="pav", bufs=2, space="PSUM"))
        pbcpool = ctx.enter_context(tc.tile_pool(name="pbc", bufs=1, space="PSUM"))

        ident = const.tile([128, 128], f32)
        make_identity(nc, ident)
        ident_b = const.tile([128, 128], bf16)
        make_identity(nc, ident_b)
        # head-pair normalize broadcast mask: row0 -> partitions 0:64,
        # row32 -> partitions 64:128 (engine writes must start at partition
        # 0/32/64/96, so the two reciprocals land on rows 0 and 32 of a
        # [33,N] tile; mask rows 1..31 are zero so those rows don't
        # contribute). Two ping-pong rinv tiles (memset once) avoid a
        # per-pair clear of the unused rows.
        mask33 = const.tile([33, 128], bf16)
        nc.vector.memset(mask33[:], 0.0)
        nc.vector.memset(mask33[0:1, 0:64], 1.0)
        nc.vector.memset(mask33[32:33, 64:128], 1.0)
        rinvA = const.tile([33, N], bf16)
        rinvB = const.tile([33, N], bf16)
        nc.vector.memset(rinvA[0:32, :], 0.0)
        nc.vector.memset(rinvB[0:32, :], 0.0)

        # ---- weights/biases declared here; emission interleaved with batch 0
        wT = const.tile([128, 6, O3], bf16)
        projT = const.tile([128, 6, C], bf16)
        qkb_sb = const.tile([128, 12], f32)
        vb_bc = const.tile([128, C], f32)
        pb_bc = const.tile([128, C], f32)

        def weights_gen():
            qb_st = stage.tile([12, 128], f32, tag="bst")
            nc.sync.dma_start(out=qb_st[:], in_=qkvb_ext[0:1536].rearrange("(j p) -> j p", p=128))
            pbt = psum.tile([128, 12], f32, tag="ps")
            nc.tensor.transpose(pbt[:], qb_st[:], ident[0:12, 0:12])
            nc.scalar.copy(out=qkb_sb[:], in_=pbt[:])

            # weight chunks: DMA fp32, cast bf16 on DVE, then 1-cycle/row PE
            # transposes. DMA+cast run two chunks ahead so the PE never waits
            # on the DMA->cast latency chain.
            wstbs = {}

            def fetch(j):
                wstf = stage.tile([128, C], f32, tag="wstf")
                src = qkvw_ext[j * 128:(j + 1) * 128, :] if j < 18 else \
                    projw_ext[(j - 18) * 128:(j - 17) * 128, :]
                nc.sync.dma_start(out=wstf[:], in_=src)
                wstb = stage.tile([128, C], bf16, tag="wstb")
                nc.vector.tensor_copy(out=wstb[:], in_=wstf[:])
                wstbs[j] = wstb

            fetch(0)
            fetch(1)
            for j in range(24):
                if j == 4:
                    nc.sync.dma_start(out=vb_bc[:], in_=qkvb_ext[1536:2304].unsqueeze(0).to_broadcast([128, C]))
                    nc.sync.dma_start(out=pb_bc[:], in_=projb_ext[:].unsqueeze(0).to_broadcast([128, C]))
                if j + 2 < 24:
                    fetch(j + 2)
                wstb = wstbs.pop(j)
                wdst = wT if j < 18 else projT
                jo = j * 128 if j < 18 else (j - 18) * 128
                # all 6 transposes into one bf16 PSUM bank, single copy out
                pt = psum.tile([128, 6 * 128], bf16, tag="ps")
                for ct in range(6):
                    nc.tensor.transpose(pt[:, ct * 128:(ct + 1) * 128],
                                        wstb[:, ct * 128:(ct + 1) * 128],
                                        ident_b[:])
                dst = wdst[:, :, jo:jo + 128]
                src_ap = pt[:].rearrange("p (g t) -> p g t", g=6)
                if j % 2 == 0:
                    nc.scalar.copy(out=dst, in_=src_ap)
                else:
                    nc.vector.tensor_copy(out=dst, in_=src_ap)
                yield

        # ---- per-rep remainder-token passes --------------------------------
        def vrem_partA(rst):
            """Remainder x rows for all 8 batches -> feature-major xT8
            [128, 6, 32] (32 = batch-major remainder tokens)."""
            xrem_f = rpool.tile([32, C], f32, tag="xrf")
            for b in range(BL):
                nc.sync.dma_start(out=xrem_f[b * 4:(b + 1) * 4, :],
                                  in_=x_ext[b, 384:388, :])
            xrem_c = rpool.tile([32, C], bf16, tag="xrc")
            nc.scalar.copy(out=xrem_c[:], in_=xrem_f[:])
            xT8 = rpool.tile([128, 6, 32], bf16, tag="xt8")
            rst["xT8"] = xT8
            pt = psum.tile([128, 192], bf16, tag="ps")
            for ct in range(6):
                nc.tensor.transpose(pt[:, ct * 32:(ct + 1) * 32],
                                    xrem_c[:, ct * 128:(ct + 1) * 128],
                                    ident_b[0:32, 0:32])
            nc.vector.tensor_copy(out=xT8[:],
                                  in_=pt[:].rearrange("p (g t) -> p g t", g=6))
            # allocate the rep's shared tiles here so stage1/proj_gen can
            # reference them before partB's matmuls are emitted
            vr8 = rpool.tile([32, H, 65], bf16, tag="vr8")
            prem8 = rpool.tile([128, 6, 32], bf16, tag="pr8")
            rst["vr8"] = vr8
            rst["prem8"] = prem8
            yield

        def vrem_partB(rst):
            """Remainder v (+ones col) for all 8 batches: vr8 [32, H, 65]."""
            xT8 = rst["xT8"]
            vr8 = rst["vr8"]
            nc.vector.memset(vr8[:, :, 64:65], 1.0)
            if "qkv" in skip:
                nc.vector.memset(vr8[:, :, 0:64], 0.0)
                return
            for o0, on, hs, he in ((0, 512, 0, 8), (512, 256, 8, 12)):
                pv = psum.tile([32, on], f32, tag="ps")
                for ct in range(6):
                    nc.tensor.matmul(
                        pv[:, 0:on],
                        lhsT=xT8[:, ct, :],
                        rhs=wT[:, ct, 1536 + o0:1536 + o0 + on],
                        start=(ct == 0), stop=(ct == 5),
                    )
                nc.vector.tensor_tensor(
                    out=vr8[:, hs:he, 0:64],
                    in0=pv[:].rearrange("p (h d) -> p h d", h=he - hs),
                    in1=vb_bc[0:32, o0:o0 + on].rearrange("p (h d) -> p h d", h=he - hs),
                    op=add,
                )
                yield

        # ---- per-batch pools ----
        xpool = ctx.enter_context(tc.tile_pool(name="xp", bufs=2))
        xtpool = ctx.enter_context(tc.tile_pool(name="xtp", bufs=2))
        qkpool = ctx.enter_context(tc.tile_pool(name="qkp", bufs=2))
        vpool = ctx.enter_context(tc.tile_pool(name="vp", bufs=2))
        apool = ctx.enter_context(tc.tile_pool(name="ap", bufs=2))
        ppool = ctx.enter_context(tc.tile_pool(name="pp", bufs=4))
        spool = ctx.enter_context(tc.tile_pool(name="ssp", bufs=4))
        opool = ctx.enter_context(tc.tile_pool(name="op", bufs=3))

        def emit_xload(b):
            # one DMA per 128-token chunk: the first transpose only waits for
            # chunk 0 (~1/3 of the full-x DMA time). Remainder rows 384:388
            # are handled by the shared vrem pass.
            xf = xpool.tile([128, 3, C], f32, tag="xf")
            for ti in range(3):
                nc.sync.dma_start(out=xf[:, ti, :],
                                  in_=x_ext[b, ti * 128:(ti + 1) * 128, :])
            return xf

        def stage1(b, xf, st, rst):
            """Generator: transposes (3 items), q/k groups (12), v (6).
            Yields between PE-work units so attention of the previous batch
            can interleave. Fills `st` with the batch's tiles."""
            xTb = xtpool.tile([128, 6, N], bf16, tag="xt")
            st["xT"] = xTb
            xc = xpool.tile([128, 3, C], bf16, tag="xc")
            for ti, (t0, tp) in enumerate(TCH3):
                # cast this chunk to bf16 on ACT, then 1-cycle/row transposes
                # (all 6 into one bf16 PSUM bank, single copy out)
                nc.scalar.copy(out=xc[0:tp, ti, :], in_=xf[0:tp, ti, :])
                pt = psum.tile([128, 6 * tp], bf16, tag="ps")
                for ct in range(6):
                    nc.tensor.transpose(pt[:, ct * tp:(ct + 1) * tp],
                                        xc[0:tp, ti, ct * 128:(ct + 1) * 128],
                                        ident_b[0:tp, 0:tp])
                dst = xTb[:, :, t0:t0 + tp]
                src_ap = pt[:].rearrange("p (g t) -> p g t", g=6)
                if ti % 2 == 0:
                    nc.vector.tensor_copy(out=dst, in_=src_ap)
                else:
                    nc.scalar.copy(out=dst, in_=src_ap)
                yield
            # remainder columns from the shared 8-batch pass
            nc.vector.tensor_copy(out=xTb[:, :, 384:388],
                                  in_=rst["xT8"][:, :, b * 4:(b + 1) * 4])

            qTb = qkpool.tile([128, 6, N], bf16, tag="q")
            kTb = qkpool.tile([128, 6, N], bf16, tag="k")
            st["q"], st["k"] = qTb, kTb
            if "qkv" in skip:
                nc.vector.memset(qTb[:, 0:1, 0:2], 0.0)
                nc.vector.memset(kTb[:, 0:1, 0:2], 0.0)
            for j in range(12 if "qkv" not in skip else 0):
                ps = psum.tile([128, N], f32, tag="ps")
                for ct in range(6):
                    nc.tensor.matmul(
                        ps[:],
                        lhsT=wT[:, ct, j * 128:(j + 1) * 128],
                        rhs=xTb[:, ct, :],
                        start=(ct == 0), stop=(ct == 5),
                    )
                dst = qTb[:, j, :] if j < 6 else kTb[:, j - 6, :]
                if j % 2 == 0:
                    nc.scalar.activation(out=dst, in_=ps[:], func=Identity,
                                         bias=qkb_sb[:, j:j + 1], scale=1.0)
                else:
                    nc.vector.tensor_scalar(out=dst, in0=ps[:],
                                            scalar1=qkb_sb[:, j:j + 1], scalar2=None,
                                            op0=add)
                yield

            # block-diagonal remainder tiles for the 4 leftover key tokens:
            # kTrem[:, cth, 0:4] = even head's k-remainder (d-rows 0:64),
            # kTrem[:, cth, 32:36] = odd head's (d-rows 64:128); other columns
            # zero so one matmul yields both heads' remainder scores.
            kTrem = qkpool.tile([128, 6, 36], bf16, tag="krem")
            st["krem"] = kTrem
            nc.vector.memset(kTrem[:], 0.0)
            nc.vector.tensor_copy(out=kTrem[0:64, :, 0:4], in_=kTb[0:64, :, 384:388])
            nc.vector.tensor_copy(out=kTrem[64:128, :, 32:36], in_=kTb[64:128, :, 384:388])

            vb = vpool.tile([128, 3, H, 65], bf16, tag="v")
            st["v"] = vb
            nc.vector.memset(vb[:, :, :, 64:65], 1.0)
            for ti, (t0, tp) in enumerate(TCH3 if "qkv" not in skip else []):
                for o0, on, hs, he in ((0, 512, 0, 8), (512, 256, 8, 12)):
                    pv = psum.tile([128, on], f32, tag="ps")
                    for ct in range(6):
                        nc.tensor.matmul(
                            pv[0:tp, 0:on],
                            lhsT=xTb[:, ct, t0:t0 + tp],
                            rhs=wT[:, ct, 1536 + o0:1536 + o0 + on],
                            start=(ct == 0), stop=(ct == 5),
                        )
                    nc.vector.tensor_tensor(
                        out=vb[0:tp, ti, hs:he, 0:64],
                        in0=pv[0:tp, :].rearrange("p (h d) -> p h d", h=he - hs),
                        in1=vb_bc[0:tp, o0:o0 + on].rearrange("p (h d) -> p h d", h=he - hs),
                        op=add,
                    )
                    yield

            # v-remainder regrouped to match kTrem's partition layout, from
            # the shared 8-batch remainder pass
            vrem = vpool.tile([36, 6, 65], bf16, tag="vrem")
            st["vrem"] = vrem
            if "qkv" not in skip:
                # DMA (no partition-alignment constraint) deinterleaves this
                # batch's remainder-v out of the shared 8-batch tile
                vr8 = rst["vr8"]
                nc.sync.dma_start(out=vrem[0:4, :, :],
                                  in_=vr8[b * 4:(b + 1) * 4, 0:12:2, :])
                nc.sync.dma_start(out=vrem[32:36, :, :],
                                  in_=vr8[b * 4:(b + 1) * 4, 1:12:2, :])
            else:
                nc.vector.memset(vrem[:], 0.0)
                nc.vector.memset(vb[:, :, :, 0:64], 0.0)

        def emit_attention(b, st, filler):
            """Attention heads; pulls filler items between scores and AVs.

            Softmax normalization is deferred and done per HEAD PAIR: the
            reciprocals of both heads' denominator rows (DVE, bf16) land in
            one [2,N] tile; a single [2,128]-mask PE matmul broadcasts them
            across the two 64-partition halves, ACT stages to SBUF, and two
            DVE multiplies normalize. This keeps gpsimd entirely out of the
            kernel and halves the broadcast/staging work vs per-head.
            """
            qTb, kTb, vb = st["q"], st["k"], st["v"]
            kTrem, vrem = st["krem"], st["vrem"]
            xattnT = apool.tile([128, 6, N], bf16, tag="xat")
            st["at"] = xattnT
            if "attn" in skip:
                for _ct in range(6):
                    nc.vector.tensor_copy(out=xattnT[:, _ct, :], in_=wT[:, 0, 0:N])
            nheads = H if "attn" not in skip else 0
            pulled = 0

            def flush_pair(pend):
                pav_e, pav_o, rinv2_p, cth_p = pend
                pbc = pbcpool.tile([128, N], f32, tag="pbc")
                nc.tensor.matmul(pbc[:], lhsT=mask33[:], rhs=rinv2_p[:],
                                 start=True, stop=True)
                # DVE can't read two PSUM operands; stage the broadcast in
                # SBUF via ACT (also keeps DVE free for the multiplies)
                rbf = spool.tile([128, N], f32, tag="rbf")
                nc.scalar.copy(out=rbf[:], in_=pbc[:])
                nc.vector.tensor_tensor(out=xattnT[0:64, cth_p, :],
                                        in0=pav_e[0:64, :], in1=rbf[0:64, :],
                                        op=mult)
                nc.vector.tensor_tensor(out=xattnT[64:128, cth_p, :],
                                        in0=pav_o[0:64, :], in1=rbf[64:128, :],
                                        op=mult)

            def emit_av(h, probs, prr, rinv2_t):
                """AV matmuls for head h (scores/exps were emitted one head
                earlier, so the exp outputs are ready — no PE stall)."""
                cth, r0 = h // 2, (h % 2) * 64
                pav = pavpool.tile([65, N], f32, tag="pav")
                nc.tensor.matmul(pav[:, 0:N], lhsT=vb[:, 0, h, :],
                                 rhs=probs[0][:, 0:N], start=True, stop=False)
                for kc, (t0, tp) in list(enumerate(TCH))[1:3]:
                    nc.tensor.matmul(pav[:, LT:N], lhsT=vb[0:tp, kc, h, :],
                                     rhs=probs[kc][0:tp, :],
                                     start=False, stop=False)
                rr = (h % 2) * 32
                nc.tensor.matmul(pav[:, LT:N], lhsT=vrem[rr:rr + 4, cth, :],
                                 rhs=prr[rr:rr + 4, :], start=False, stop=True)

                if "epi" in skip:
                    nc.vector.tensor_copy(out=xattnT[r0:r0 + 64, cth, :],
                                          in_=pav[0:64, :])
                    return None
                r32 = (h % 2) * 32
                with nc.allow_low_precision(reason="bf16 1/denom feeds a "
                                            "mask-broadcast matmul; ~0.2% "
                                            "rel err, budget is 2e-2"):
                    nc.vector.reciprocal(out=rinv2_t[r32:r32 + 1, :],
                                         in_=pav[64:65, :])
                return pav

            pend = None    # completed head pair awaiting normalize
            prev = None    # (h, probs, prr) awaiting AV for head h-1
            rinv2_t = None
            stash = None   # even head's pav awaiting its pair
            for h in range(nheads):
                cth, r0 = h // 2, (h % 2) * 64
                qh = qTb[r0:r0 + 64, cth, :]   # [64, 388] bf16
                kh = kTb[r0:r0 + 64, cth, :]

                # all scores matmuls first; chunk 0 covers ALL queries
                # (template cols 0:128 + search 128:388); the 4 remainder
                # keys are computed for the HEAD PAIR at even h via the
                # block-diagonal kTrem in one matmul + one exp
                probs = []
                for kc, (t0, tp) in list(enumerate(TCH))[0:3]:
                    pss = psum.tile([128, N if kc == 0 else LS], f32, tag="ps")
                    rhs_q = qh[:] if kc == 0 else qh[:, LT:N]
                    nc.tensor.matmul(pss[0:tp, :], lhsT=kh[:, t0:t0 + tp],
                                     rhs=rhs_q, start=True, stop=True)
                    prs = ppool.tile([128, N if kc == 0 else LS], bf16,
                                     tag="pr0" if kc == 0 else "prs")
                    nc.scalar.activation(out=prs[0:tp, :], in_=pss[0:tp, :],
                                         func=(Identity if "expid" in skip else Exp),
                                         scale=SCALE)
                    probs.append(prs)
                if h % 2 == 0:
                    psr = psum.tile([36, LS], f32, tag="ps")
                    nc.tensor.matmul(psr[:], lhsT=kTrem[:, cth, :],
                                     rhs=qTb[:, cth, LT:N], start=True, stop=True)
                    prr = ppool.tile([36, LS], bf16, tag="prr")
                    nc.scalar.activation(out=prr[:], in_=psr[:],
                                         func=(Identity if "expid" in skip else Exp),
                                         scale=SCALE)
                    st["prr"] = prr
                else:
                    prr = st["prr"]

                # filler work for neighbouring batches rides in the exp window
                want = (h + 1) * 22 // nheads
                while pulled < want and next(filler, "END") != "END":
                    pulled += 1

                # normalize the pair completed two heads ago, then AV for
                # head h-1: both consume results whose producers ran during
                # this head's scores
                if pend is not None and h % 2 == 1:
                    flush_pair(pend)
                    pend = None
                if prev is not None:
                    hp = prev[0]
                    if hp % 2 == 0:
                        rinv2_t = rinvA if (hp // 2) % 2 == 0 else rinvB
                        stash = emit_av(*prev, rinv2_t)
                    else:
                        pav_o = emit_av(*prev, rinv2_t)
                        if stash is not None and pav_o is not None:
                            pend = (stash, pav_o, rinv2_t, hp // 2)
                prev = (h, probs, prr)

            # tail: AV of the last head completes the final pair
            pend2 = None
            if prev is not None:
                hp = prev[0]
                pav_o = emit_av(*prev, rinv2_t)
                if stash is not None and pav_o is not None:
                    pend2 = (stash, pav_o, rinv2_t, hp // 2)
            if pend is not None:
                flush_pair(pend)
            # drain any remaining filler, then normalize the last pair
            while next(filler, "END") != "END":
                pass
            if "attn" not in skip and pend2 is not None:
                flush_pair(pend2)

        def proj_gen(b, st, rst, last):
            """Generator: 3 full proj chunk items; remainder tokens are
            staged into the rep-shared prem8 tile and projected for all 8
            batches at once after the rep's last batch."""
            if "proj" in skip:
                return
            xattnT = st["at"]
            for ti, (t0, tp) in enumerate(TCH3):
                osb = opool.tile([128, C], f32, tag="ob")
                for o0, on in ((0, 512), (512, 256)):
                    pp = psum.tile([128, on], f32, tag="ps")
                    for ct in range(6):
                        nc.tensor.matmul(
                            pp[0:tp, 0:on],
                            lhsT=xattnT[:, ct, t0:t0 + tp],
                            rhs=projT[:, ct, o0:o0 + on],
                            start=(ct == 0), stop=(ct == 5),
                        )
                    nc.vector.tensor_tensor(out=osb[0:tp, o0:o0 + on], in0=pp[0:tp, :],
                                            in1=pb_bc[0:tp, o0:o0 + on], op=add)
                nc.sync.dma_start(out=out_ext[b, t0:t0 + tp, :], in_=osb[0:tp, :])
                yield
            # stage this batch's remainder columns into the rep-shared tile
            prem8 = rst["prem8"]
            nc.vector.tensor_copy(out=prem8[:, :, b * 4:(b + 1) * 4],
                                  in_=xattnT[:, :, 384:388])
            yield
            if last:
                osb = opool.tile([32, C], f32, tag="obr")
                for o0, on in ((0, 512), (512, 256)):
                    pp = psum.tile([32, on], f32, tag="ps")
                    for ct in range(6):
                        nc.tensor.matmul(
                            pp[:, 0:on],
                            lhsT=prem8[:, ct, :],
                            rhs=projT[:, ct, o0:o0 + on],
                            start=(ct == 0), stop=(ct == 5),
                        )
                    nc.vector.tensor_tensor(out=osb[:, o0:o0 + on], in0=pp[:],
                                            in1=pb_bc[0:32, o0:o0 + on], op=add)
                for bb in range(BL):
                    nc.sync.dma_start(out=out_ext[bb, 384:388, :],
                                      in_=osb[bb * 4:(bb + 1) * 4, :])
                yield

        # ---- software-pipelined batch loop ----
        from itertools import chain

        seq = [bb for _ in range(reps) for bb in range(BL)]
        states = [dict() for _ in seq]
        rstates = [dict() for _ in range(reps)]

        # rep-0 remainder partA first: its tiny DMAs+cast+transposes complete
        # while the first big x-chunk DMA is still in flight, so the PE's
        # first real work is not delayed.
        for _ in vrem_partA(rstates[0]):
            pass
        gw = weights_gen()
        xf0 = emit_xload(seq[0])
        g0 = stage1(seq[0], xf0, states[0], rstates[0])
        gB = vrem_partB(rstates[0])
        # prologue: interleave the 24 weight-prep chunks with batch 0's
        # stage1 items (21: 3 transposes + 12 qk + 6 v). Transposes need no
        # weights; qk group j (item 3+j) needs weight chunk j; v items
        # (idx 15..20) need weight chunks 12-17. Give gw a 2-chunk lead
        # after the transposes so every consumer finds its weights already
        # emitted in PE program order (gw call m completes chunks 0..m-2).
        for i in range(3):
            next(g0, None)
            next(gw, None)
        next(gw, None)
        next(gw, None)
        for i in range(18):
            next(gw, None)
            next(g0, None)
        for _ in gw:
            pass
        # partB (produces vr8) must be emitted before g0's tail, which
        # copies this batch's vrem slices out of vr8.
        for _ in gB:
            pass
        for _ in g0:
            pass

        prev_proj = iter(())
        for i, b in enumerate(seq):
            r = i // BL
            extra = iter(())
            if i % BL == BL - 1 and r + 1 < reps:
                rstn = rstates[r + 1]
                extra = chain(vrem_partA(rstn), vrem_partB(rstn))
            if i + 1 < len(seq):
                xf_n = emit_xload(seq[i + 1])
                nxt = stage1(seq[i + 1], xf_n, states[i + 1],
                             rstates[(i + 1) // BL])
            else:
                nxt = iter(())
            emit_attention(b, states[i], chain(prev_proj, extra, nxt))
            prev_proj = proj_gen(b, states[i], rstates[r],
                                 last=(i % BL == BL - 1))
        for _ in prev_proj:
            pass

    nc.compile()
    return nc


def _get_nc():
    if "nc" not in _NC_CACHE:
        _NC_CACHE["nc"] = _build_nc()
    return _NC_CACHE["nc"]


def kernel(x, qkv_w, qkv_b, proj_w, proj_b, t_h=8, t_w=8, s_h=16, s_w=16):
    from concourse.bass_utils import run_bass_kernel_spmd

    x = np.ascontiguousarray(np.asarray(x, dtype=np.float32))
    qkv_w = np.ascontiguousarray(np.asarray(qkv_w, dtype=np.float32))
    qkv_b = np.ascontiguousarray(np.asarray(qkv_b, dtype=np.float32))
    proj_w = np.ascontiguousarray(np.asarray(proj_w, dtype=np.float32))
    proj_b = np.ascontiguousarray(np.asarray(proj_b, dtype=np.float32))

    nc = _get_nc()
    in_maps = [
        {
            "x": x[i * BL:(i + 1) * BL],
            "qkv_w": qkv_w,
            "qkv_b": qkv_b,
            "proj_w": proj_w,
            "proj_b": proj_b,
        }
        for i in range(NCORES)
    ]
    res = run_bass_kernel_spmd(nc, in_maps, core_ids=list(range(NCORES)))
    out = np.concatenate([res.results[i]["out"] for i in range(NCORES)], axis=0)
    return out.astype(np.float32)
